# revision 11
# baseline (speedup 1.0000x reference)
"""Trainium2 Bass kernel for nn_BlockConvolution_1 (gnn_message_passing).

Math restructuring (verified exact vs reference):
  support = input @ W; per crop c: blk_c = adj[:, s:e, s:e] @ support[:, s:e, :]
  BatchNorm of the zero-padded blk_c means rows outside crop c contribute just
  beta_c, so with j = n // 10:
      out[b, n, f] = alpha[n, f] * blk[b, n, f] + e[n, f]
      alpha = gamma_diag * rsqrt(var + eps)
      e     = beta_eff - alpha * mean          (beta_eff = sum of all betas)
  where blk = blockdiag(adj) @ input @ W and mean/var are per-(n, f) batch
  statistics of blk over the full batch B.

Device mapping (8 cores, data-parallel over batch, 1024 (+2 pad) per core):
  groups of 3 batches -> 120 rows.  Pass 1 per group:
    T^T[fin, rows] = matmul(lhsT=input_rows[120, 128fin], rhs=adjT_blockdiag)
    blk[rows, f]   = matmul(lhsT=T^T[128, 120], rhs=W[128, 256]) over 2 chunks
  blk is cast to bf16 into a persistent SBUF cache [120, NCACHE, 256]; stats
  (sum / sum-of-squares) accumulate in PSUM via one-hot matmuls.  AllReduce of
  [40, 512] sums across cores; alpha/e computed on device.  Pass 2 is a pure
  DVE affine over the cache (out = cache * alpha + e, bf16) streamed to DRAM;
  TAIL groups that don't fit the cache are recomputed (overlapping the
  collective) and affined from PSUM.
"""

import numpy as np
import ml_dtypes

B, NN, FIN, FOUT = 8192, 40, 256, 256
NBLK, BLK = 4, 10
NC = 8
BPC = B // NC          # 1024 batches per core
GB = 3                 # batches per group (3*40 = 120 rows <= 128 contraction)
GROUPS = 342           # ceil(1024/3) -> padded to 1026 batches
BPCP = GROUPS * GB     # 1026
ROWS = GB * NN         # 120
PAIRS = GROUPS // 2    # 171
SBG = 8                # groups per superblock (input DMA batching)
NCACHE = 342           # groups cached in SBUF (rest recomputed in pass 2)
EPS = 1e-5

_BF16 = ml_dtypes.bfloat16
_CACHE = {}


# ----------------------------------------------------------------------------
# device program
# ----------------------------------------------------------------------------
def _build_program():
    from contextlib import ExitStack
    from concourse import bass, bacc, tile

    mybir = bass.mybir
    dt = mybir.dt
    AF = mybir.ActivationFunctionType

    nc = bacc.Bacc("TRN2", target_bir_lowering=False, debug=False, num_devices=NC)

    xin = nc.dram_tensor("xin", [ROWS, GROUPS, FIN], dt.bfloat16, kind="ExternalInput").ap()
    adt = nc.dram_tensor("adt", [ROWS, GROUPS, ROWS], dt.bfloat16, kind="ExternalInput").ap()
    wp = nc.dram_tensor("wp", [128, 2, FOUT], dt.bfloat16, kind="ExternalInput").ap()
    son = nc.dram_tensor("son", [ROWS, NN], dt.bfloat16, kind="ExternalInput").ap()
    gdi = nc.dram_tensor("gdi", [NN, FOUT], dt.float32, kind="ExternalInput").ap()
    bef = nc.dram_tensor("bef", [NN, FOUT], dt.float32, kind="ExternalInput").ap()
    out = nc.dram_tensor("out", [ROWS, GROUPS, FOUT], dt.bfloat16, kind="ExternalOutput").ap()

    NTAIL = GROUPS - NCACHE
    assert NTAIL == 0

    with tile.TileContext(nc) as tc, ExitStack() as ctx:
        const = ctx.enter_context(tc.tile_pool(name="const", bufs=1))
        cache_p = ctx.enter_context(tc.tile_pool(name="cachep", bufs=1))
        inp_p = ctx.enter_context(tc.tile_pool(name="inp", bufs=2))
        adt_p = ctx.enter_context(tc.tile_pool(name="adtp", bufs=2))
        ttps_p = ctx.enter_context(tc.tile_pool(name="ttps", bufs=2, space="PSUM"))
        ttsb_p = ctx.enter_context(tc.tile_pool(name="ttsb", bufs=2))
        blk_p = ctx.enter_context(tc.tile_pool(name="blkps", bufs=2, space="PSUM"))
        sq_p = ctx.enter_context(tc.tile_pool(name="sqp", bufs=2))
        fold_p = ctx.enter_context(tc.tile_pool(name="foldp", bufs=2))
        stat_p = ctx.enter_context(tc.tile_pool(name="statps", bufs=1, space="PSUM"))
        smal_p = ctx.enter_context(tc.tile_pool(name="small", bufs=1))
        dram_p = ctx.enter_context(tc.tile_pool(name="dram", bufs=1, space="DRAM"))

        # constants (explicit tags: untagged tiles in a pool share one slot)
        wp_t = const.tile([128, 2, FOUT], dt.bfloat16, tag="wp")
        nc.sync.dma_start(out=wp_t[:], in_=wp[:])
        son_t = const.tile([ROWS, NN], dt.bfloat16, tag="son")
        nc.sync.dma_start(out=son_t[:], in_=son[:])
        gdi_t = const.tile([NN, FOUT], dt.float32, tag="gdi")
        nc.sync.dma_start(out=gdi_t[:], in_=gdi[:])
        bef_t = const.tile([NN, FOUT], dt.float32, tag="bef")
        nc.sync.dma_start(out=bef_t[:], in_=bef[:])

        alpha4 = const.tile([ROWS, 4, FOUT], dt.bfloat16, tag="alpha4")
        e4 = const.tile([ROWS, 4, FOUT], dt.bfloat16, tag="e4")

        cache = cache_p.tile([ROWS, NCACHE, FOUT], dt.bfloat16, tag="cache")

        sum_ps = stat_p.tile([NN, FOUT], dt.float32, tag="sum")
        sq_ps = stat_p.tile([NN, FOUT], dt.float32, tag="sq")
        warm_ps = stat_p.tile([128, FOUT], dt.float32, tag="warm")

        # initial PE warm-up: a short dense burst so HAM reaches K=8/8
        for _ in range(32):
            nc.tensor.matmul(warm_ps[:], wp_t[:, 0, 0:128], wp_t[:, 1, :],
                             start=True, stop=True)

        def load_sb(sb0, nsb):
            xt = inp_p.tile([ROWS, SBG, FIN], dt.bfloat16, tag="xt")
            at = adt_p.tile([ROWS, SBG, ROWS], dt.bfloat16, tag="at")
            nc.sync.dma_start(out=xt[:, 0:nsb, :], in_=xin[:, sb0:sb0 + nsb, :])
            nc.sync.dma_start(out=at[:, 0:nsb, :], in_=adt[:, sb0:sb0 + nsb, :])
            return xt, at

        def pair_matmuls(xt, at, q):
            """A + W matmuls for pair q (groups 2q, 2q+1 of the superblock)."""
            gA = 2 * q
            ttp = ttps_p.tile([128, 4, ROWS], dt.float32, tag="ttp")
            for g2 in range(2):
                for c in range(2):
                    nc.tensor.matmul(
                        ttp[:, 2 * g2 + c, :],
                        xt[:, gA + g2, c * 128:(c + 1) * 128],
                        at[:, gA + g2, :],
                        start=True, stop=True,
                    )
            tts = ttsb_p.tile([128, 4, ROWS], dt.bfloat16, tag="tts")
            nc.vector.tensor_copy(tts[:, 0:1, :], ttp[:, 0:1, :])
            nc.scalar.activation(tts[:, 1:4, :], ttp[:, 1:4, :], AF.Copy)
            bps = blk_p.tile([ROWS, 2, FOUT], dt.float32, tag="bps")
            for g2 in range(2):
                for c in range(2):
                    nc.tensor.matmul(
                        bps[:, g2, :],
                        tts[:, 2 * g2 + c, :],
                        wp_t[:, c, :],
                        start=(c == 0), stop=(c == 1),
                    )
            return bps

        # ---------------- pass 1: blk -> cache + stats ----------------
        pair_idx = 0
        for sb0 in range(0, NCACHE, SBG):
            nsb = min(SBG, NCACHE - sb0)
            xt, at = load_sb(sb0, nsb)
            for q in range(nsb // 2):
                gA = sb0 + 2 * q
                bps = pair_matmuls(xt, at, q)
                # blk -> persistent bf16 cache (DVE), squares (ACT) for stats
                nc.vector.tensor_copy(cache[:, gA:gA + 2, :], bps[:, :, :])
                sqt = sq_p.tile([ROWS, 2, FOUT], dt.bfloat16, tag="sqt")
                nc.scalar.activation(sqt[:], bps[:, :, :], AF.Square)
                # pair-fold on GpSimd halves the stats matmul column count
                sfold = fold_p.tile([ROWS, FOUT], dt.bfloat16, tag="sfold")
                qfold = fold_p.tile([ROWS, FOUT], dt.bfloat16, tag="qfold")
                nc.gpsimd.tensor_add(sfold[:], cache[:, gA, :], cache[:, gA + 1, :])
                nc.gpsimd.tensor_add(qfold[:], sqt[:, 0, :], sqt[:, 1, :])
                nc.tensor.matmul(sum_ps[:, :], son_t[:], sfold[:],
                                 start=(pair_idx == 0), stop=(pair_idx == PAIRS - 1))
                nc.tensor.matmul(sq_ps[:, :], son_t[:], qfold[:],
                                 start=(pair_idx == 0), stop=(pair_idx == PAIRS - 1))
                pair_idx += 1
        assert pair_idx == PAIRS

        # ---- stats: AllReduce, compute alpha & e ----
        cc_sb = smal_p.tile([NN, 2 * FOUT], dt.float32, tag="ccsb")
        scr = smal_p.tile([NN, 2, FOUT], dt.float32, tag="scr")
        nc.vector.tensor_copy(cc_sb[:, 0:FOUT], sum_ps[:])
        nc.scalar.activation(cc_sb[:, FOUT:2 * FOUT], sq_ps[:], AF.Copy)
        cc_in = dram_p.tile([NN, 2 * FOUT], dt.float32, tag="ccin")
        cc_out = dram_p.tile([NN, 2 * FOUT], dt.float32, tag="ccout")
        nc.sync.dma_start(out=cc_in[:], in_=cc_sb[:])
        nc.gpsimd.collective_compute(
            "AllReduce",
            mybir.AluOpType.add,
            replica_groups=[list(range(NC))],
            ins=[cc_in.opt()],
            outs=[cc_out.opt()],
        )
        all_sb = smal_p.tile([NN, 2 * FOUT], dt.float32, tag="allsb")
        nc.sync.dma_start(out=all_sb[:], in_=cc_out[:])

        mean = smal_p.tile([NN, FOUT], dt.float32, tag="mean")
        xv = smal_p.tile([NN, FOUT], dt.float32, tag="xv")
        r0 = smal_p.tile([NN, FOUT], dt.float32, tag="r0")
        alph = smal_p.tile([NN, FOUT], dt.float32, tag="alph")
        # scr/cc_sb are dead after the collective; reuse as scratch
        t1 = scr[:, 0, :]
        t2 = scr[:, 1, :]
        r1 = cc_sb[:, 0:FOUT]
        e40 = cc_sb[:, FOUT:2 * FOUT]

        nc.vector.tensor_scalar_mul(mean[:], all_sb[:, 0:FOUT], 1.0 / B)
        nc.vector.tensor_scalar_mul(xv[:], all_sb[:, FOUT:2 * FOUT], 1.0 / B)
        nc.vector.tensor_mul(t1[:], mean[:], mean[:])
        nc.vector.tensor_sub(t2[:], xv[:], t1[:])                  # var
        nc.vector.tensor_scalar_add(xv[:], t2[:], EPS)             # var + eps
        nc.scalar.activation(t1[:], xv[:], AF.Sqrt)
        nc.vector.reciprocal(r0[:], t1[:])                         # ~rsqrt
        for _ in range(2):                                         # Newton refine
            nc.vector.tensor_mul(t1[:], r0[:], r0[:])
            nc.vector.tensor_mul(t2[:], t1[:], xv[:])
            nc.vector.tensor_scalar(t1[:], t2[:], -0.5, 1.5,
                                    mybir.AluOpType.mult, mybir.AluOpType.add)
            nc.vector.tensor_mul(r1[:], r0[:], t1[:])
            r0, r1 = r1, r0
        nc.vector.tensor_mul(alph[:], gdi_t[:], r0[:])
        nc.vector.tensor_mul(t1[:], alph[:], mean[:])
        nc.vector.tensor_sub(e40[:], bef_t[:], t1[:])              # e = bef - a*mean

        for h in range(4):
            nc.vector.tensor_copy(alpha4[0:NN, h, :], alph[:])
            nc.scalar.activation(e4[0:NN, h, :], e40[:], AF.Copy)
        for m in range(1, GB):
            nc.sync.dma_start(out=alpha4[m * NN:(m + 1) * NN, :, :],
                              in_=alpha4[0:NN, :, :])
            nc.sync.dma_start(out=e4[m * NN:(m + 1) * NN, :, :],
                              in_=e4[0:NN, :, :])

        # ---------------- pass 2: in-place affine on the cache ----------------
        SPAN = 16  # groups per output DMA (8 KB per partition line)
        ci = 0
        for sb0 in range(0, NCACHE, SPAN):
            nsb = min(SPAN, NCACHE - sb0)
            for c0 in range(sb0, sb0 + nsb, 4):
                no = min(4, sb0 + nsb - c0)
                cs = cache[:, c0:c0 + no, :]
                nc.vector.tensor_mul(cs, cs, alpha4[:, 0:no, :])
                # every third add goes to GpSimd to unload DVE
                if ci % 3 == 2:
                    nc.gpsimd.tensor_add(cs, cs, e4[:, 0:no, :])
                else:
                    nc.vector.tensor_add(cs, cs, e4[:, 0:no, :])
                ci += 1
            nc.sync.dma_start(out=out[:, sb0:sb0 + nsb, :],
                              in_=cache[:, sb0:sb0 + nsb, :])

    nc.compile()
    return nc


# ----------------------------------------------------------------------------
# runner: shard_map over 8 cores with pre-placed device inputs
# ----------------------------------------------------------------------------
def _get_exec():
    if "exec" in _CACHE:
        return _CACHE["exec"]

    import jax
    import jax.numpy as jnp
    from jax.experimental.shard_map import shard_map
    from jax.sharding import Mesh, PartitionSpec, NamedSharding
    from concourse import bass2jax, mybir

    nc = _build_program()
    _CACHE["nc"] = nc
    bass2jax.install_neuronx_cc_hook()

    partition_name = nc.partition_id_tensor.name if nc.partition_id_tensor else None
    in_names, out_names, out_avals = [], [], []
    for alloc in nc.m.functions[0].allocations:
        if not isinstance(alloc, mybir.MemoryLocationSet):
            continue
        name = alloc.memorylocations[0].name
        if alloc.kind == "ExternalInput":
            if name != partition_name:
                in_names.append(name)
        elif alloc.kind == "ExternalOutput":
            out_names.append(name)
            out_avals.append(
                jax.core.ShapedArray(tuple(alloc.tensor_shape), mybir.dt.np(alloc.dtype))
            )
    n_params = len(in_names)
    n_outs = len(out_names)
    all_names = in_names + out_names
    if partition_name is not None:
        all_names = all_names + [partition_name]

    def _body(*args):
        operands = list(args)
        if partition_name is not None:
            operands.append(bass2jax.partition_id_tensor())
        outs = bass2jax._bass_exec_p.bind(
            *operands,
            out_avals=tuple(out_avals),
            in_names=tuple(all_names),
            out_names=tuple(out_names),
            lowering_input_output_aliases=(),
            sim_require_finite=True,
            sim_require_nnan=True,
            nc=nc,
        )
        return tuple(outs)

    devices = jax.devices()[:NC]
    mesh = Mesh(np.asarray(devices), ("core",))
    in_specs = (PartitionSpec("core"),) * (n_params + n_outs)
    out_specs = (PartitionSpec("core"),) * n_outs
    donate = tuple(range(n_params, n_params + n_outs))
    sharded = jax.jit(
        shard_map(_body, mesh=mesh, in_specs=in_specs, out_specs=out_specs,
                  check_rep=False),
        donate_argnums=donate, keep_unused=True,
    )
    sharding = NamedSharding(mesh, PartitionSpec("core"))

    zero_fns = []
    for av in out_avals:
        gshape = (NC * av.shape[0], *av.shape[1:])
        zero_fns.append(jax.jit(
            lambda shp=gshape, dt=av.dtype: jnp.zeros(shp, dt),
            out_shardings=sharding,
        ))

    _CACHE["exec"] = (sharded, sharding, in_names, out_names, out_avals, zero_fns)
    return _CACHE["exec"]


def _device_inputs(in_maps):
    """Concat per-core inputs and place on the mesh (outside the NEFF)."""
    import jax
    sharded, sharding, in_names, out_names, out_avals, zero_fns = _get_exec()
    concat = [
        np.ascontiguousarray(np.concatenate([m[name] for m in in_maps], axis=0))
        for name in in_names
    ]
    dev_in = [jax.device_put(a, sharding) for a in concat]
    for a in dev_in:
        a.block_until_ready()
    return dev_in


def _run_once(dev_in):
    import jax
    sharded, sharding, in_names, out_names, out_avals, zero_fns = _get_exec()
    zeros = [f() for f in zero_fns]
    for z in zeros:
        z.block_until_ready()
    outs = sharded(*dev_in, *zeros)
    res = {}
    for name, av, arr in zip(out_names, out_avals, outs):
        res[name] = np.asarray(arr).reshape(NC, *av.shape)
    return res


# ----------------------------------------------------------------------------
# host data prep
# ----------------------------------------------------------------------------
def _prep_core(inp_c, adj_c):
    """inp_c [1024, 40, 256] f32, adj_c [1024, 40, 40] f32 -> xin, adt arrays."""
    ip = np.zeros((BPCP, NN, FIN), np.float32)
    ip[:BPC] = inp_c
    # [g, m, n, f] -> partition-major [ (m,n)=120, g, f ]
    xin = np.ascontiguousarray(
        ip.reshape(GROUPS, GB, NN, FIN).transpose(1, 2, 0, 3).reshape(ROWS, GROUPS, FIN)
    ).astype(_BF16)

    ap = np.zeros((BPCP, NN, NN), np.float32)
    ap[:BPC] = adj_c
    A = ap.reshape(GROUPS, GB, NN, NN)
    bd = np.zeros((GROUPS, ROWS, ROWS), np.float32)
    for m in range(GB):
        for c in range(NBLK):
            s = c * BLK
            o = m * NN + s
            bd[:, o:o + BLK, o:o + BLK] = A[:, m, s:s + BLK, s:s + BLK].transpose(0, 2, 1)
    adt = np.ascontiguousarray(bd.transpose(1, 0, 2)).astype(_BF16)
    return xin, adt


def _prep_shared(W, bn_gamma, bn_beta):
    wp = np.ascontiguousarray(
        np.stack([W[0:128, :], W[128:256, :]], axis=1)
    ).astype(_BF16)                                            # [128, 2, 256]
    son = np.ascontiguousarray(np.tile(np.eye(NN, dtype=np.float32), (GB, 1))).astype(_BF16)
    G = np.asarray(bn_gamma, np.float32).reshape(NBLK, NN, FOUT)
    nidx = np.arange(NN)
    gdi = np.ascontiguousarray(G[nidx // BLK, nidx, :])
    bef = np.ascontiguousarray(np.asarray(bn_beta, np.float32).reshape(NBLK, NN, FOUT).sum(axis=0))
    return wp, son, gdi, bef


def kernel(input, adj, W, bn_gamma, bn_beta):
    input = np.asarray(input, np.float32)
    adj = np.asarray(adj, np.float32)
    W = np.asarray(W, np.float32)
    wp, son, gdi, bef = _prep_shared(W, bn_gamma, bn_beta)

    in_maps = []
    for c in range(NC):
        sl = slice(c * BPC, (c + 1) * BPC)
        xin, adt = _prep_core(input[sl], adj[sl])
        in_maps.append({
            "xin": xin, "adt": adt, "wp": wp, "son": son,
            "gdi": gdi, "bef": bef,
        })

    dev_in = _device_inputs(in_maps)
    res = _run_once(dev_in)

    outs = []
    for c in range(NC):
        o = np.asarray(res["out"][c], np.float32)           # [120, 342, 256]
        o = o.reshape(GB, NN, GROUPS, FOUT).transpose(2, 0, 1, 3).reshape(BPCP, NN, FOUT)
        outs.append(o[:BPC])
    return np.ascontiguousarray(np.concatenate(outs, axis=0))


# revision 20
# speedup vs baseline: 1.3686x; 1.3686x over previous
"""Trainium2 Bass kernel for nn_BlockConvolution_1 (gnn_message_passing).

Math restructuring (verified exact vs reference):
  support = input @ W; per crop c: blk_c = adj[:, s:e, s:e] @ support[:, s:e, :]
  BatchNorm of the zero-padded blk_c means rows outside crop c contribute just
  beta_c, so with j = n // 10:
      out[b, n, f] = alpha[n, f] * blk[b, n, f] + e[n, f]
      alpha = gamma_diag * rsqrt(var + eps)
      e     = beta_eff - alpha * mean          (beta_eff = sum of all betas)
  where blk = blockdiag(adj) @ input @ W and mean/var are per-(n, f) batch
  statistics of blk over the full batch B.

Device mapping (8 cores, data-parallel over batch, 1024 (+2 pad) per core):
  groups of 3 batches -> 120 rows.  Pass 1 per group:
    T^T[fin, rows] = matmul(lhsT=input_rows[120, 128fin], rhs=adjT_blockdiag)
    blk[rows, f]   = matmul(lhsT=T^T[128, 120], rhs=W[128, 256]) over 2 chunks
  blk is cast to bf16 into a persistent SBUF cache [120, NCACHE, 256]; stats
  (sum / sum-of-squares) accumulate in PSUM via one-hot matmuls.  AllReduce of
  [40, 512] sums across cores; alpha/e computed on device.  Pass 2 is a pure
  DVE affine over the cache (out = cache * alpha + e, bf16) streamed to DRAM;
  TAIL groups that don't fit the cache are recomputed (overlapping the
  collective) and affined from PSUM.
"""

import numpy as np
import ml_dtypes

B, NN, FIN, FOUT = 8192, 40, 256, 256
NBLK, BLK = 4, 10
NC = 8
BPC = B // NC          # 1024 batches per core
GB = 3                 # batches per group (3*40 = 120 rows <= 128 contraction)
GROUPS = 342           # ceil(1024/3) -> padded to 1026 batches
BPCP = GROUPS * GB     # 1026
ROWS = GB * NN         # 120
PAIRS = GROUPS // 2    # 171
SBG = 6                # groups per superblock (input DMA batching)
NCACHE = 342           # groups cached in SBUF (rest recomputed in pass 2)
EPS = 1e-5

_BF16 = ml_dtypes.bfloat16
_CACHE = {}


# ----------------------------------------------------------------------------
# device program
# ----------------------------------------------------------------------------
def _build_program():
    from contextlib import ExitStack
    from concourse import bass, bacc, tile

    mybir = bass.mybir
    dt = mybir.dt
    AF = mybir.ActivationFunctionType

    nc = bacc.Bacc("TRN2", target_bir_lowering=False, debug=False, num_devices=NC)

    xin = nc.dram_tensor("xin", [ROWS, GROUPS, FIN], dt.bfloat16, kind="ExternalInput").ap()
    adt = nc.dram_tensor("adt", [ROWS, GROUPS, ROWS], dt.bfloat16, kind="ExternalInput").ap()
    wp = nc.dram_tensor("wp", [128, 2, FOUT], dt.bfloat16, kind="ExternalInput").ap()
    son = nc.dram_tensor("son", [ROWS, NN], dt.bfloat16, kind="ExternalInput").ap()
    gdi = nc.dram_tensor("gdi", [NN, FOUT], dt.float32, kind="ExternalInput").ap()
    bef = nc.dram_tensor("bef", [NN, FOUT], dt.float32, kind="ExternalInput").ap()
    out = nc.dram_tensor("out", [ROWS, GROUPS, FOUT], dt.bfloat16, kind="ExternalOutput").ap()

    NTAIL = GROUPS - NCACHE
    assert NTAIL == 0

    with tile.TileContext(nc) as tc, ExitStack() as ctx:
        const = ctx.enter_context(tc.tile_pool(name="const", bufs=1))
        cache_p = ctx.enter_context(tc.tile_pool(name="cachep", bufs=1))
        inp_p = ctx.enter_context(tc.tile_pool(name="inp", bufs=3))
        adt_p = ctx.enter_context(tc.tile_pool(name="adtp", bufs=3))
        ttps_p = ctx.enter_context(tc.tile_pool(name="ttps", bufs=3, space="PSUM"))
        ttsb_p = ctx.enter_context(tc.tile_pool(name="ttsb", bufs=3))
        blk_p = ctx.enter_context(tc.tile_pool(name="blkps", bufs=3, space="PSUM"))
        sq_p = ctx.enter_context(tc.tile_pool(name="sqp", bufs=2))
        outs_p = ctx.enter_context(tc.tile_pool(name="outsp", bufs=3))
        stat_p = ctx.enter_context(tc.tile_pool(name="statps", bufs=1, space="PSUM"))
        smal_p = ctx.enter_context(tc.tile_pool(name="small", bufs=1))
        dram_p = ctx.enter_context(tc.tile_pool(name="dram", bufs=1, space="DRAM"))

        # constants (explicit tags: untagged tiles in a pool share one slot)
        wp_t = const.tile([128, 2, FOUT], dt.bfloat16, tag="wp")
        nc.sync.dma_start(out=wp_t[:], in_=wp[:])
        son_t = const.tile([ROWS, NN], dt.bfloat16, tag="son")
        nc.sync.dma_start(out=son_t[:], in_=son[:])
        gdi_t = const.tile([NN, FOUT], dt.float32, tag="gdi")
        nc.sync.dma_start(out=gdi_t[:], in_=gdi[:])
        bef_t = const.tile([NN, FOUT], dt.float32, tag="bef")
        nc.sync.dma_start(out=bef_t[:], in_=bef[:])

        alpha2 = const.tile([ROWS, 2, FOUT], dt.bfloat16, tag="alpha2")
        e2 = const.tile([ROWS, 2, FOUT], dt.bfloat16, tag="e2")

        cache = cache_p.tile([ROWS, NCACHE, FOUT], dt.bfloat16, tag="cache")

        sum_ps = stat_p.tile([NN, 2, FOUT], dt.float32, tag="sum")
        sq_ps = stat_p.tile([NN, 2, FOUT], dt.float32, tag="sq")

        # initial PE warm-up: a dense burst so the p-state ramps to 2.4 GHz.
        # Writes a ttp-pool slot (recycled by the real A-matmuls afterwards)
        # to keep all 8 PSUM banks available for the pipeline.
        warm = ttps_p.tile([128, 4, ROWS], dt.float32, tag="ttp")
        for _ in range(16):
            nc.tensor.matmul(warm[:, 0:2, :], wp_t[:, 0, 0:128],
                             wp_t[:, 1, 0:2 * ROWS], start=True, stop=True)

        def load_sb(sb0, nsb):
            xt = inp_p.tile([ROWS, SBG, FIN], dt.bfloat16, tag="xt")
            at = adt_p.tile([ROWS, SBG, ROWS], dt.bfloat16, tag="at")
            nc.sync.dma_start(out=xt[:, 0:nsb, :], in_=xin[:, sb0:sb0 + nsb, :])
            nc.sync.dma_start(out=at[:, 0:nsb, :], in_=adt[:, sb0:sb0 + nsb, :])
            return xt, at

        def pair_matmuls(xt, at, q):
            """A + W matmuls for pair q (groups 2q, 2q+1 of the superblock)."""
            gA = 2 * q
            ttp = ttps_p.tile([128, 4, ROWS], dt.float32, tag="ttp")
            for g2 in range(2):
                for c in range(2):
                    nc.tensor.matmul(
                        ttp[:, 2 * g2 + c, :],
                        xt[:, gA + g2, c * 128:(c + 1) * 128],
                        at[:, gA + g2, :],
                        start=True, stop=True,
                    )
            tts = ttsb_p.tile([128, 4, ROWS], dt.bfloat16, tag="tts")
            nc.vector.tensor_copy(tts[:, 0:1, :], ttp[:, 0:1, :])
            nc.scalar.activation(tts[:, 1:4, :], ttp[:, 1:4, :], AF.Copy)
            bps = blk_p.tile([ROWS, 2, FOUT], dt.float32, tag="bps")
            for g2 in range(2):
                for c in range(2):
                    nc.tensor.matmul(
                        bps[:, g2, :],
                        tts[:, 2 * g2 + c, :],
                        wp_t[:, c, :],
                        start=(c == 0), stop=(c == 1),
                    )
            return bps

        # ---------------- pass 1: blk -> cache + stats ----------------
        pair_idx = 0
        for sb0 in range(0, NCACHE, SBG):
            nsb = min(SBG, NCACHE - sb0)
            xt, at = load_sb(sb0, nsb)
            for q in range(nsb // 2):
                gA = sb0 + 2 * q
                bps = pair_matmuls(xt, at, q)
                # blk -> persistent bf16 cache (DVE), squares (ACT) for stats
                nc.vector.tensor_copy(cache[:, gA:gA + 2, :], bps[:, :, :])
                sqt = sq_p.tile([ROWS, 2, FOUT], dt.bfloat16, tag="sqt")
                nc.scalar.activation(sqt[:], bps[:, :, :], AF.Square)
                nc.tensor.matmul(sum_ps[:, :, :], son_t[:], cache[:, gA:gA + 2, :],
                                 start=(pair_idx == 0), stop=(pair_idx == PAIRS - 1))
                nc.tensor.matmul(sq_ps[:, :, :], son_t[:], sqt[:],
                                 start=(pair_idx == 0), stop=(pair_idx == PAIRS - 1))
                pair_idx += 1
        assert pair_idx == PAIRS

        # ---- stats: fold pair halves, AllReduce, compute alpha & e ----
        cc_sb = smal_p.tile([NN, 2 * FOUT], dt.float32, tag="ccsb")
        scr = smal_p.tile([NN, 2, FOUT], dt.float32, tag="scr")
        nc.vector.tensor_copy(scr[:], sum_ps[:])
        nc.vector.tensor_add(cc_sb[:, 0:FOUT], scr[:, 0, :], scr[:, 1, :])
        nc.scalar.activation(scr[:], sq_ps[:], AF.Copy)
        nc.vector.tensor_add(cc_sb[:, FOUT:2 * FOUT], scr[:, 0, :], scr[:, 1, :])
        cc_in = dram_p.tile([NN, 2 * FOUT], dt.float32, tag="ccin")
        cc_out = dram_p.tile([NN, 2 * FOUT], dt.float32, tag="ccout")
        nc.sync.dma_start(out=cc_in[:], in_=cc_sb[:])
        nc.gpsimd.collective_compute(
            "AllReduce",
            mybir.AluOpType.add,
            replica_groups=[list(range(NC))],
            ins=[cc_in.opt()],
            outs=[cc_out.opt()],
        )
        # collective result lands in scr (dead after the folds above)
        nc.sync.dma_start(out=scr[:], in_=cc_out[:])

        mean = smal_p.tile([NN, FOUT], dt.float32, tag="mean")
        xv = smal_p.tile([NN, FOUT], dt.float32, tag="xv")
        r0 = smal_p.tile([NN, FOUT], dt.float32, tag="r0")
        # cc_sb is dead after the collective consumed it; reuse as scratch
        t1 = cc_sb[:, 0:FOUT]
        t2 = cc_sb[:, FOUT:2 * FOUT]

        nc.vector.tensor_scalar_mul(mean[:], scr[:, 0, :], 1.0 / B)
        nc.vector.tensor_scalar_mul(xv[:], scr[:, 1, :], 1.0 / B)
        nc.vector.tensor_mul(t1[:], mean[:], mean[:])
        nc.vector.tensor_sub(t2[:], xv[:], t1[:])                  # var
        nc.vector.tensor_scalar_add(xv[:], t2[:], EPS)             # var + eps
        nc.scalar.activation(t1[:], xv[:], AF.Sqrt)
        nc.vector.reciprocal(r0[:], t1[:])                         # ~rsqrt
        r1 = scr[:, 0, :]                                          # scr dead now
        for _ in range(2):                                         # Newton refine
            nc.vector.tensor_mul(t1[:], r0[:], r0[:])
            nc.vector.tensor_mul(t2[:], t1[:], xv[:])
            nc.vector.tensor_scalar(t1[:], t2[:], -0.5, 1.5,
                                    mybir.AluOpType.mult, mybir.AluOpType.add)
            nc.vector.tensor_mul(r1[:], r0[:], t1[:])
            r0, r1 = r1, r0
        alph = scr[:, 1, :]
        e40 = xv  # var+eps dead after rsqrt
        nc.vector.tensor_mul(alph[:], gdi_t[:], r0[:])
        nc.vector.tensor_mul(t1[:], alph[:], mean[:])
        nc.vector.tensor_sub(e40[:], bef_t[:], t1[:])              # e = bef - a*mean

        for h in range(2):
            nc.vector.tensor_copy(alpha2[0:NN, h, :], alph[:])
            nc.scalar.activation(e2[0:NN, h, :], e40[:], AF.Copy)
        for m in range(1, GB):
            nc.sync.dma_start(out=alpha2[m * NN:(m + 1) * NN, :, :],
                              in_=alpha2[0:NN, :, :])
            nc.sync.dma_start(out=e2[m * NN:(m + 1) * NN, :, :],
                              in_=e2[0:NN, :, :])

        # ---------------- pass 2: affine cache -> staging -> DRAM ----------------
        SBO = 4  # groups per output DMA block (2-group affine ops)
        for sb0 in range(0, NCACHE, SBO):
            nsb = min(SBO, NCACHE - sb0)
            ot = outs_p.tile([ROWS, SBO, FOUT], dt.bfloat16, tag="ot")
            for q2 in range(0, nsb, 2):
                osl = ot[:, q2:q2 + 2, :]
                nc.vector.tensor_mul(osl, cache[:, sb0 + q2:sb0 + q2 + 2, :],
                                     alpha2[:, :, :])
                nc.vector.tensor_add(osl, osl, e2[:, :, :])
            nc.sync.dma_start(out=out[:, sb0:sb0 + nsb, :], in_=ot[:, 0:nsb, :])

    nc.compile()
    return nc


# ----------------------------------------------------------------------------
# runner: shard_map over 8 cores with pre-placed device inputs
# ----------------------------------------------------------------------------
def _get_exec():
    if "exec" in _CACHE:
        return _CACHE["exec"]

    import jax
    import jax.numpy as jnp
    from jax.experimental.shard_map import shard_map
    from jax.sharding import Mesh, PartitionSpec, NamedSharding
    from concourse import bass2jax, mybir

    nc = _build_program()
    _CACHE["nc"] = nc
    bass2jax.install_neuronx_cc_hook()

    partition_name = nc.partition_id_tensor.name if nc.partition_id_tensor else None
    in_names, out_names, out_avals = [], [], []
    for alloc in nc.m.functions[0].allocations:
        if not isinstance(alloc, mybir.MemoryLocationSet):
            continue
        name = alloc.memorylocations[0].name
        if alloc.kind == "ExternalInput":
            if name != partition_name:
                in_names.append(name)
        elif alloc.kind == "ExternalOutput":
            out_names.append(name)
            out_avals.append(
                jax.core.ShapedArray(tuple(alloc.tensor_shape), mybir.dt.np(alloc.dtype))
            )
    n_params = len(in_names)
    n_outs = len(out_names)
    all_names = in_names + out_names
    if partition_name is not None:
        all_names = all_names + [partition_name]

    def _body(*args):
        operands = list(args)
        if partition_name is not None:
            operands.append(bass2jax.partition_id_tensor())
        outs = bass2jax._bass_exec_p.bind(
            *operands,
            out_avals=tuple(out_avals),
            in_names=tuple(all_names),
            out_names=tuple(out_names),
            lowering_input_output_aliases=(),
            sim_require_finite=True,
            sim_require_nnan=True,
            nc=nc,
        )
        return tuple(outs)

    devices = jax.devices()[:NC]
    mesh = Mesh(np.asarray(devices), ("core",))
    in_specs = (PartitionSpec("core"),) * (n_params + n_outs)
    out_specs = (PartitionSpec("core"),) * n_outs
    donate = tuple(range(n_params, n_params + n_outs))
    sharded = jax.jit(
        shard_map(_body, mesh=mesh, in_specs=in_specs, out_specs=out_specs,
                  check_rep=False),
        donate_argnums=donate, keep_unused=True,
    )
    sharding = NamedSharding(mesh, PartitionSpec("core"))

    zero_fns = []
    for av in out_avals:
        gshape = (NC * av.shape[0], *av.shape[1:])
        zero_fns.append(jax.jit(
            lambda shp=gshape, dt=av.dtype: jnp.zeros(shp, dt),
            out_shardings=sharding,
        ))

    _CACHE["exec"] = (sharded, sharding, in_names, out_names, out_avals, zero_fns)
    return _CACHE["exec"]


def _device_inputs(in_maps):
    """Concat per-core inputs and place on the mesh (outside the NEFF)."""
    import jax
    sharded, sharding, in_names, out_names, out_avals, zero_fns = _get_exec()
    concat = [
        np.ascontiguousarray(np.concatenate([m[name] for m in in_maps], axis=0))
        for name in in_names
    ]
    dev_in = [jax.device_put(a, sharding) for a in concat]
    for a in dev_in:
        a.block_until_ready()
    return dev_in


def _run_once(dev_in):
    import jax
    sharded, sharding, in_names, out_names, out_avals, zero_fns = _get_exec()
    zeros = [f() for f in zero_fns]
    for z in zeros:
        z.block_until_ready()
    outs = sharded(*dev_in, *zeros)
    res = {}
    for name, av, arr in zip(out_names, out_avals, outs):
        res[name] = np.asarray(arr).reshape(NC, *av.shape)
    return res


# ----------------------------------------------------------------------------
# host data prep
# ----------------------------------------------------------------------------
def _prep_core(inp_c, adj_c):
    """inp_c [1024, 40, 256] f32, adj_c [1024, 40, 40] f32 -> xin, adt arrays."""
    ip = np.zeros((BPCP, NN, FIN), np.float32)
    ip[:BPC] = inp_c
    # [g, m, n, f] -> partition-major [ (m,n)=120, g, f ]
    xin = np.ascontiguousarray(
        ip.reshape(GROUPS, GB, NN, FIN).transpose(1, 2, 0, 3).reshape(ROWS, GROUPS, FIN)
    ).astype(_BF16)

    ap = np.zeros((BPCP, NN, NN), np.float32)
    ap[:BPC] = adj_c
    A = ap.reshape(GROUPS, GB, NN, NN)
    bd = np.zeros((GROUPS, ROWS, ROWS), np.float32)
    for m in range(GB):
        for c in range(NBLK):
            s = c * BLK
            o = m * NN + s
            bd[:, o:o + BLK, o:o + BLK] = A[:, m, s:s + BLK, s:s + BLK].transpose(0, 2, 1)
    adt = np.ascontiguousarray(bd.transpose(1, 0, 2)).astype(_BF16)
    return xin, adt


def _prep_shared(W, bn_gamma, bn_beta):
    wp = np.ascontiguousarray(
        np.stack([W[0:128, :], W[128:256, :]], axis=1)
    ).astype(_BF16)                                            # [128, 2, 256]
    son = np.ascontiguousarray(np.tile(np.eye(NN, dtype=np.float32), (GB, 1))).astype(_BF16)
    G = np.asarray(bn_gamma, np.float32).reshape(NBLK, NN, FOUT)
    nidx = np.arange(NN)
    gdi = np.ascontiguousarray(G[nidx // BLK, nidx, :])
    bef = np.ascontiguousarray(np.asarray(bn_beta, np.float32).reshape(NBLK, NN, FOUT).sum(axis=0))
    return wp, son, gdi, bef


def kernel(input, adj, W, bn_gamma, bn_beta):
    input = np.asarray(input, np.float32)
    adj = np.asarray(adj, np.float32)
    W = np.asarray(W, np.float32)
    wp, son, gdi, bef = _prep_shared(W, bn_gamma, bn_beta)

    in_maps = []
    for c in range(NC):
        sl = slice(c * BPC, (c + 1) * BPC)
        xin, adt = _prep_core(input[sl], adj[sl])
        in_maps.append({
            "xin": xin, "adt": adt, "wp": wp, "son": son,
            "gdi": gdi, "bef": bef,
        })

    dev_in = _device_inputs(in_maps)
    res = _run_once(dev_in)

    outs = []
    for c in range(NC):
        o = np.asarray(res["out"][c], np.float32)           # [120, 342, 256]
        o = o.reshape(GB, NN, GROUPS, FOUT).transpose(2, 0, 1, 3).reshape(BPCP, NN, FOUT)
        outs.append(o[:BPC])
    return np.ascontiguousarray(np.concatenate(outs, axis=0))


# revision 27
# speedup vs baseline: 1.4559x; 1.0637x over previous
"""Trainium2 Bass kernel for nn_BlockConvolution_1 (gnn_message_passing).

Math restructuring (verified exact vs reference):
  support = input @ W; per crop c: blk_c = adj[:, s:e, s:e] @ support[:, s:e, :]
  BatchNorm of the zero-padded blk_c means rows outside crop c contribute just
  beta_c, so with j = n // 10:
      out[b, n, f] = alpha[n, f] * blk[b, n, f] + e[n, f]
      alpha = gamma_diag * rsqrt(var + eps)
      e     = beta_eff - alpha * mean          (beta_eff = sum of all betas)
  where blk = blockdiag(adj) @ input @ W and mean/var are per-(n, f) batch
  statistics of blk over the full batch B.

Device mapping (8 cores, data-parallel over batch, 1024 (+2 pad) per core):
  groups of 3 batches -> 120 rows.  Pass 1 per group:
    T^T[fin, rows] = matmul(lhsT=input_rows[120, 128fin], rhs=adjT_blockdiag)
    blk[rows, f]   = matmul(lhsT=T^T[128, 120], rhs=W[128, 256]) over 2 chunks
  blk is cast to bf16 into a persistent SBUF cache [120, NCACHE, 256]; stats
  (sum / sum-of-squares) accumulate in PSUM via one-hot matmuls.  AllReduce of
  [40, 512] sums across cores; alpha/e computed on device.  Pass 2 is a pure
  DVE affine over the cache (out = cache * alpha + e, bf16) streamed to DRAM;
  TAIL groups that don't fit the cache are recomputed (overlapping the
  collective) and affined from PSUM.
"""

import numpy as np
import ml_dtypes

B, NN, FIN, FOUT = 8192, 40, 256, 256
NBLK, BLK = 4, 10
NC = 8
BPC = B // NC          # 1024 batches per core
GB = 3                 # batches per group (3*40 = 120 rows <= 128 contraction)
GROUPS = 342           # ceil(1024/3) -> padded to 1026 batches
BPCP = GROUPS * GB     # 1026
ROWS = GB * NN         # 120
ROWSP = 128            # rows padded to 128 (full PE stationary => fast weight load)
PAIRS = GROUPS // 2    # 171
SBG = 6                # groups per superblock (input DMA batching)
NCACHE = 342           # groups cached in SBUF (rest recomputed in pass 2)
EPS = 1e-5

_BF16 = ml_dtypes.bfloat16
_CACHE = {}


# ----------------------------------------------------------------------------
# device program
# ----------------------------------------------------------------------------
def _build_program():
    from contextlib import ExitStack
    from concourse import bass, bacc, tile

    mybir = bass.mybir
    dt = mybir.dt
    AF = mybir.ActivationFunctionType

    nc = bacc.Bacc("TRN2", target_bir_lowering=False, debug=False, num_devices=NC)

    xin = nc.dram_tensor("xin", [ROWSP, GROUPS, FIN], dt.bfloat16, kind="ExternalInput").ap()
    adt = nc.dram_tensor("adt", [ROWSP, GROUPS, ROWSP], dt.bfloat16, kind="ExternalInput").ap()
    wp = nc.dram_tensor("wp", [128, 2, FOUT], dt.bfloat16, kind="ExternalInput").ap()
    son = nc.dram_tensor("son", [ROWSP, NN], dt.bfloat16, kind="ExternalInput").ap()
    gdi = nc.dram_tensor("gdi", [NN, FOUT], dt.float32, kind="ExternalInput").ap()
    bef = nc.dram_tensor("bef", [NN, FOUT], dt.float32, kind="ExternalInput").ap()
    out = nc.dram_tensor("out", [ROWS, GROUPS, FOUT], dt.bfloat16, kind="ExternalOutput").ap()

    NTAIL = GROUPS - NCACHE
    assert NTAIL == 0

    with tile.TileContext(nc) as tc, ExitStack() as ctx:
        const = ctx.enter_context(tc.tile_pool(name="const", bufs=1))
        cache_p = ctx.enter_context(tc.tile_pool(name="cachep", bufs=1))
        inp_p = ctx.enter_context(tc.tile_pool(name="inp", bufs=3))
        adt_p = ctx.enter_context(tc.tile_pool(name="adtp", bufs=3))
        ttps_p = ctx.enter_context(tc.tile_pool(name="ttps", bufs=3, space="PSUM"))
        ttsb_p = ctx.enter_context(tc.tile_pool(name="ttsb", bufs=3))
        blk_p = ctx.enter_context(tc.tile_pool(name="blkps", bufs=3, space="PSUM"))
        sq_p = ctx.enter_context(tc.tile_pool(name="sqp", bufs=2))
        outs_p = ctx.enter_context(tc.tile_pool(name="outsp", bufs=3))
        stat_p = ctx.enter_context(tc.tile_pool(name="statps", bufs=1, space="PSUM"))
        smal_p = ctx.enter_context(tc.tile_pool(name="small", bufs=1))
        dram_p = ctx.enter_context(tc.tile_pool(name="dram", bufs=1, space="DRAM"))

        # constants (explicit tags: untagged tiles in a pool share one slot)
        wp_t = const.tile([128, 2, FOUT], dt.bfloat16, tag="wp")
        nc.sync.dma_start(out=wp_t[:], in_=wp[:])
        son_t = const.tile([ROWSP, NN], dt.bfloat16, tag="son")
        nc.sync.dma_start(out=son_t[:], in_=son[:])
        gdi_t = const.tile([NN, FOUT], dt.float32, tag="gdi")
        nc.sync.dma_start(out=gdi_t[:], in_=gdi[:])
        bef_t = const.tile([NN, FOUT], dt.float32, tag="bef")
        nc.sync.dma_start(out=bef_t[:], in_=bef[:])

        alpha2 = const.tile([ROWSP, 2, FOUT], dt.bfloat16, tag="alpha2")
        e2 = const.tile([ROWSP, 2, FOUT], dt.bfloat16, tag="e2")
        # zero once so pad rows 120..127 stay finite for the padded affine ops
        nc.vector.memset(alpha2[:], 0.0)
        nc.scalar.memzero(e2[:])

        cache = cache_p.tile([ROWSP, NCACHE, FOUT], dt.bfloat16, tag="cache")

        sum_ps = stat_p.tile([NN, 2, FOUT], dt.float32, tag="sum")
        sq_ps = stat_p.tile([NN, 2, FOUT], dt.float32, tag="sq")

        # initial PE warm-up: a dense burst so the p-state ramps to 2.4 GHz.
        # Writes a ttp-pool slot (recycled by the real A-matmuls afterwards)
        # to keep all 8 PSUM banks available for the pipeline.
        warm = ttps_p.tile([128, 4, ROWSP], dt.float32, tag="ttp")
        for _ in range(16):
            nc.tensor.matmul(warm[:, 0:2, :], wp_t[:, 0, 0:128],
                             wp_t[:, 1, 0:2 * ROWSP], start=True, stop=True)

        def load_sb(sb0, nsb):
            xt = inp_p.tile([ROWSP, SBG, FIN], dt.bfloat16, tag="xt")
            at = adt_p.tile([ROWSP, SBG, ROWSP], dt.bfloat16, tag="at")
            nc.sync.dma_start(out=xt[:, 0:nsb, :], in_=xin[:, sb0:sb0 + nsb, :])
            nc.sync.dma_start(out=at[:, 0:nsb, :], in_=adt[:, sb0:sb0 + nsb, :])
            return xt, at

        def pair_matmuls(xt, at, q):
            """A + W matmuls for pair q (groups 2q, 2q+1 of the superblock)."""
            gA = 2 * q
            ttp = ttps_p.tile([128, 4, ROWSP], dt.float32, tag="ttp")
            for g2 in range(2):
                for c in range(2):
                    nc.tensor.matmul(
                        ttp[:, 2 * g2 + c, :],
                        xt[:, gA + g2, c * 128:(c + 1) * 128],
                        at[:, gA + g2, :],
                        start=True, stop=True,
                    )
            tts = ttsb_p.tile([128, 4, ROWSP], dt.bfloat16, tag="tts")
            nc.vector.tensor_copy(tts[:, 0:1, :], ttp[:, 0:1, :])
            nc.scalar.activation(tts[:, 1:4, :], ttp[:, 1:4, :], AF.Copy)
            bps = blk_p.tile([ROWSP, 2, FOUT], dt.float32, tag="bps")
            for g2 in range(2):
                for c in range(2):
                    nc.tensor.matmul(
                        bps[:, g2, :],
                        tts[:, 2 * g2 + c, :],
                        wp_t[:, c, :],
                        start=(c == 0), stop=(c == 1),
                    )
            return bps

        # ---------------- pass 1: blk -> cache + stats ----------------
        pair_idx = 0
        for sb0 in range(0, NCACHE, SBG):
            nsb = min(SBG, NCACHE - sb0)
            xt, at = load_sb(sb0, nsb)
            for q in range(nsb // 2):
                gA = sb0 + 2 * q
                bps = pair_matmuls(xt, at, q)
                # blk -> persistent bf16 cache (DVE), squares (ACT) for stats
                nc.vector.tensor_copy(cache[:, gA:gA + 2, :], bps[:, :, :])
                sqt = sq_p.tile([ROWSP, 2, FOUT], dt.bfloat16, tag="sqt")
                nc.scalar.activation(sqt[:], bps[:, :, :], AF.Square)
                nc.tensor.matmul(sum_ps[:, :, :], son_t[:], cache[:, gA:gA + 2, :],
                                 start=(pair_idx == 0), stop=(pair_idx == PAIRS - 1))
                nc.tensor.matmul(sq_ps[:, :, :], son_t[:], sqt[:],
                                 start=(pair_idx == 0), stop=(pair_idx == PAIRS - 1))
                pair_idx += 1
        assert pair_idx == PAIRS

        # ---- stats: fold pair halves, AllReduce, compute alpha & e ----
        cc_sb = smal_p.tile([NN, 2 * FOUT], dt.float32, tag="ccsb")
        scr = smal_p.tile([NN, 2, FOUT], dt.float32, tag="scr")
        nc.vector.tensor_copy(scr[:], sum_ps[:])
        nc.vector.tensor_add(cc_sb[:, 0:FOUT], scr[:, 0, :], scr[:, 1, :])
        nc.scalar.activation(scr[:], sq_ps[:], AF.Copy)
        nc.vector.tensor_add(cc_sb[:, FOUT:2 * FOUT], scr[:, 0, :], scr[:, 1, :])
        cc_in = dram_p.tile([NN, 2 * FOUT], dt.float32, tag="ccin")
        cc_out = dram_p.tile([NN, 2 * FOUT], dt.float32, tag="ccout")
        nc.sync.dma_start(out=cc_in[:], in_=cc_sb[:])
        nc.gpsimd.collective_compute(
            "AllReduce",
            mybir.AluOpType.add,
            replica_groups=[list(range(NC))],
            ins=[cc_in.opt()],
            outs=[cc_out.opt()],
        )
        # collective result lands in scr (dead after the folds above)
        nc.sync.dma_start(out=scr[:], in_=cc_out[:])

        mean = smal_p.tile([NN, FOUT], dt.float32, tag="mean")
        xv = smal_p.tile([NN, FOUT], dt.float32, tag="xv")
        r0 = smal_p.tile([NN, FOUT], dt.float32, tag="r0")
        # cc_sb is dead after the collective consumed it; reuse as scratch
        t1 = cc_sb[:, 0:FOUT]
        t2 = cc_sb[:, FOUT:2 * FOUT]

        nc.vector.tensor_scalar_mul(mean[:], scr[:, 0, :], 1.0 / B)
        nc.vector.tensor_scalar_mul(xv[:], scr[:, 1, :], 1.0 / B)
        nc.vector.tensor_mul(t1[:], mean[:], mean[:])
        nc.vector.tensor_sub(t2[:], xv[:], t1[:])                  # var
        nc.vector.tensor_scalar_add(xv[:], t2[:], EPS)             # var + eps
        nc.scalar.activation(t1[:], xv[:], AF.Sqrt)
        nc.vector.reciprocal(r0[:], t1[:])                         # ~rsqrt
        r1 = scr[:, 0, :]                                          # scr dead now
        for _ in range(2):                                         # Newton refine
            nc.vector.tensor_mul(t1[:], r0[:], r0[:])
            nc.vector.tensor_mul(t2[:], t1[:], xv[:])
            nc.vector.tensor_scalar(t1[:], t2[:], -0.5, 1.5,
                                    mybir.AluOpType.mult, mybir.AluOpType.add)
            nc.vector.tensor_mul(r1[:], r0[:], t1[:])
            r0, r1 = r1, r0
        alph = scr[:, 1, :]
        e40 = xv  # var+eps dead after rsqrt
        nc.vector.tensor_mul(alph[:], gdi_t[:], r0[:])
        nc.vector.tensor_mul(t1[:], alph[:], mean[:])
        nc.vector.tensor_sub(e40[:], bef_t[:], t1[:])              # e = bef - a*mean

        for h in range(2):
            nc.vector.tensor_copy(alpha2[0:NN, h, :], alph[:])
            nc.scalar.activation(e2[0:NN, h, :], e40[:], AF.Copy)
        for m in range(1, GB):
            nc.sync.dma_start(out=alpha2[m * NN:(m + 1) * NN, :, :],
                              in_=alpha2[0:NN, :, :])
            nc.sync.dma_start(out=e2[m * NN:(m + 1) * NN, :, :],
                              in_=e2[0:NN, :, :])
        # ---------------- pass 2: affine cache -> staging -> DRAM ----------------
        SBO = 4  # groups per output DMA block (2-group affine ops)
        bi = 0
        for sb0 in range(0, NCACHE, SBO):
            nsb = min(SBO, NCACHE - sb0)
            ot = outs_p.tile([ROWSP, SBO, FOUT], dt.bfloat16, tag="ot")
            for q2 in range(0, nsb, 2):
                osl = ot[:, q2:q2 + 2, :]
                nc.vector.tensor_mul(osl, cache[:, sb0 + q2:sb0 + q2 + 2, :],
                                     alpha2[:, :, :])
                # every 4th add on GpSimd to unload DVE
                if bi % 4 == 3:
                    nc.gpsimd.tensor_add(osl, osl, e2[:, :, :])
                else:
                    nc.vector.tensor_add(osl, osl, e2[:, :, :])
                bi += 1
            nc.sync.dma_start(out=out[:, sb0:sb0 + nsb, :], in_=ot[0:ROWS, 0:nsb, :])

    nc.compile()
    return nc


# ----------------------------------------------------------------------------
# runner: shard_map over 8 cores with pre-placed device inputs
# ----------------------------------------------------------------------------
def _get_exec():
    if "exec" in _CACHE:
        return _CACHE["exec"]

    import jax
    import jax.numpy as jnp
    from jax.experimental.shard_map import shard_map
    from jax.sharding import Mesh, PartitionSpec, NamedSharding
    from concourse import bass2jax, mybir

    nc = _build_program()
    _CACHE["nc"] = nc
    bass2jax.install_neuronx_cc_hook()

    partition_name = nc.partition_id_tensor.name if nc.partition_id_tensor else None
    in_names, out_names, out_avals = [], [], []
    for alloc in nc.m.functions[0].allocations:
        if not isinstance(alloc, mybir.MemoryLocationSet):
            continue
        name = alloc.memorylocations[0].name
        if alloc.kind == "ExternalInput":
            if name != partition_name:
                in_names.append(name)
        elif alloc.kind == "ExternalOutput":
            out_names.append(name)
            out_avals.append(
                jax.core.ShapedArray(tuple(alloc.tensor_shape), mybir.dt.np(alloc.dtype))
            )
    n_params = len(in_names)
    n_outs = len(out_names)
    all_names = in_names + out_names
    if partition_name is not None:
        all_names = all_names + [partition_name]

    def _body(*args):
        operands = list(args)
        if partition_name is not None:
            operands.append(bass2jax.partition_id_tensor())
        outs = bass2jax._bass_exec_p.bind(
            *operands,
            out_avals=tuple(out_avals),
            in_names=tuple(all_names),
            out_names=tuple(out_names),
            lowering_input_output_aliases=(),
            sim_require_finite=True,
            sim_require_nnan=True,
            nc=nc,
        )
        return tuple(outs)

    devices = jax.devices()[:NC]
    mesh = Mesh(np.asarray(devices), ("core",))
    in_specs = (PartitionSpec("core"),) * (n_params + n_outs)
    out_specs = (PartitionSpec("core"),) * n_outs
    donate = tuple(range(n_params, n_params + n_outs))
    sharded = jax.jit(
        shard_map(_body, mesh=mesh, in_specs=in_specs, out_specs=out_specs,
                  check_rep=False),
        donate_argnums=donate, keep_unused=True,
    )
    sharding = NamedSharding(mesh, PartitionSpec("core"))

    zero_fns = []
    for av in out_avals:
        gshape = (NC * av.shape[0], *av.shape[1:])
        zero_fns.append(jax.jit(
            lambda shp=gshape, dt=av.dtype: jnp.zeros(shp, dt),
            out_shardings=sharding,
        ))

    _CACHE["exec"] = (sharded, sharding, in_names, out_names, out_avals, zero_fns)
    return _CACHE["exec"]


def _device_inputs(in_maps):
    """Concat per-core inputs and place on the mesh (outside the NEFF)."""
    import jax
    sharded, sharding, in_names, out_names, out_avals, zero_fns = _get_exec()
    concat = [
        np.ascontiguousarray(np.concatenate([m[name] for m in in_maps], axis=0))
        for name in in_names
    ]
    dev_in = [jax.device_put(a, sharding) for a in concat]
    for a in dev_in:
        a.block_until_ready()
    return dev_in


def _run_once(dev_in):
    import jax
    sharded, sharding, in_names, out_names, out_avals, zero_fns = _get_exec()
    zeros = [f() for f in zero_fns]
    for z in zeros:
        z.block_until_ready()
    outs = sharded(*dev_in, *zeros)
    res = {}
    for name, av, arr in zip(out_names, out_avals, outs):
        res[name] = np.asarray(arr).reshape(NC, *av.shape)
    return res


# ----------------------------------------------------------------------------
# host data prep
# ----------------------------------------------------------------------------
def _prep_core(inp_c, adj_c):
    """inp_c [1024, 40, 256] f32, adj_c [1024, 40, 40] f32 -> xin, adt arrays."""
    ip = np.zeros((BPCP, NN, FIN), np.float32)
    ip[:BPC] = inp_c
    # [g, m, n, f] -> partition-major [ (m,n)=120 (+8 pad), g, f ]
    xin = np.zeros((ROWSP, GROUPS, FIN), np.float32)
    xin[:ROWS] = ip.reshape(GROUPS, GB, NN, FIN).transpose(1, 2, 0, 3).reshape(
        ROWS, GROUPS, FIN)
    xin = np.ascontiguousarray(xin).astype(_BF16)

    ap = np.zeros((BPCP, NN, NN), np.float32)
    ap[:BPC] = adj_c
    A = ap.reshape(GROUPS, GB, NN, NN)
    bd = np.zeros((GROUPS, ROWSP, ROWSP), np.float32)
    for m in range(GB):
        for c in range(NBLK):
            s = c * BLK
            o = m * NN + s
            bd[:, o:o + BLK, o:o + BLK] = A[:, m, s:s + BLK, s:s + BLK].transpose(0, 2, 1)
    adt = np.ascontiguousarray(bd.transpose(1, 0, 2)).astype(_BF16)
    return xin, adt


def _prep_shared(W, bn_gamma, bn_beta):
    wp = np.ascontiguousarray(
        np.stack([W[0:128, :], W[128:256, :]], axis=1)
    ).astype(_BF16)                                            # [128, 2, 256]
    son = np.zeros((ROWSP, NN), np.float32)
    son[:ROWS] = np.tile(np.eye(NN, dtype=np.float32), (GB, 1))
    son = np.ascontiguousarray(son).astype(_BF16)
    G = np.asarray(bn_gamma, np.float32).reshape(NBLK, NN, FOUT)
    nidx = np.arange(NN)
    gdi = np.ascontiguousarray(G[nidx // BLK, nidx, :])
    bef = np.ascontiguousarray(np.asarray(bn_beta, np.float32).reshape(NBLK, NN, FOUT).sum(axis=0))
    return wp, son, gdi, bef


def kernel(input, adj, W, bn_gamma, bn_beta):
    input = np.asarray(input, np.float32)
    adj = np.asarray(adj, np.float32)
    W = np.asarray(W, np.float32)
    wp, son, gdi, bef = _prep_shared(W, bn_gamma, bn_beta)

    in_maps = []
    for c in range(NC):
        sl = slice(c * BPC, (c + 1) * BPC)
        xin, adt = _prep_core(input[sl], adj[sl])
        in_maps.append({
            "xin": xin, "adt": adt, "wp": wp, "son": son,
            "gdi": gdi, "bef": bef,
        })

    dev_in = _device_inputs(in_maps)
    res = _run_once(dev_in)

    outs = []
    for c in range(NC):
        o = np.asarray(res["out"][c], np.float32)           # [120, 342, 256]
        o = o.reshape(GB, NN, GROUPS, FOUT).transpose(2, 0, 1, 3).reshape(BPCP, NN, FOUT)
        outs.append(o[:BPC])
    return np.ascontiguousarray(np.concatenate(outs, axis=0))
